# revision 33
# baseline (speedup 1.0000x reference)
"""Bahdanau attention Trainium2 kernel.

Computation (per batch b):
  q[u]     = (hidden[b] @ W2)[u] + b2[u] + b1[u]
  projT    = W1.T @ feat[b].T            -> [U, S]   (PE, bf16)
  t        = tanh(projT + q[:, None])    -> [U, S]   (ACT, fused per-partition bias)
  score[s] = sum_u V[u] * t[u, s]                    (PE, tanh-slices as stationary)
  w[s]     = exp(score[s])               (no max-sub: |score| <= sum|V| ~ 13, safe in f32)
  total    = sum_s w[s]                              (ACT accum_out + ones-matmul)
  aw       = w / total
  ctx[d]   = sum_s w[s] * feat[b,s,d] / total        (PE, w-columns as stationary)

Data-parallel over 8 NeuronCores: batch dim 64 -> 8 per core, weights replicated.
"""

import os
import numpy as np

B, S, D, H, U = 64, 4096, 256, 512, 256
NCORES = 8
BC = B // NCORES          # batches per core
SCHUNK = 1024             # s rows per chunk
NCHUNK = S // SCHUNK      # 4
NSUB = SCHUNK // 128      # 8 sub-tiles of 128 rows per chunk
NCOL = S // 128           # 32 score columns per batch

_CACHE = {}


def _build_nc():
    import concourse.bass as bass  # noqa: F401
    import concourse.mybir as mybir
    from concourse import bacc, tile
    import ml_dtypes

    f32 = mybir.dt.float32
    bf16 = mybir.dt.bfloat16
    AF = mybir.ActivationFunctionType

    nc = bacc.Bacc("TRN2")

    feats = nc.declare_dram_parameter("features", [BC, S, D], f32, isOutput=False)
    # qT = (hidden @ W2 + b1 + b2).T, precomputed host-side (tiny)
    qt_in = nc.declare_dram_parameter("qT", [U, BC], f32, isOutput=False)
    w1 = nc.declare_dram_parameter("W1", [D, U], f32, isOutput=False)
    v = nc.declare_dram_parameter("V", [U, 1], f32, isOutput=False)
    ctx_out = nc.declare_dram_parameter("ctx", [BC, D], f32, isOutput=True)
    aw_out = nc.declare_dram_parameter("aw", [BC, S, 1], f32, isOutput=True)

    ident_f32_dram = nc.inline_tensor(np.eye(128, dtype=np.float32), name="ident_f32")
    ident_bf_dram = nc.inline_tensor(
        np.eye(128).astype(ml_dtypes.bfloat16), name="ident_bf"
    )
    ones_col_dram = nc.inline_tensor(np.ones((128, 1), np.float32), name="ones_col")
    ones_row_dram = nc.inline_tensor(np.ones((1, 128), np.float32), name="ones_row")

    with tile.TileContext(nc) as tc:
        with (
            tc.tile_pool(name="const", bufs=1) as cpool,
            tc.tile_pool(name="fb", bufs=NCHUNK + 4) as fb_pool,
            tc.tile_pool(name="ftt", bufs=8) as ftt_pool,
            tc.tile_pool(name="th", bufs=6) as th_pool,
            tc.tile_pool(name="small", bufs=3) as sm_pool,
            tc.tile_pool(name="ftt_ps", bufs=2, space="PSUM") as ftt_ps_pool,
            tc.tile_pool(name="pf_ps", bufs=2, space="PSUM") as pf_ps_pool,
            tc.tile_pool(name="sc_ps", bufs=1, space="PSUM") as sc_ps_pool,
            tc.tile_pool(name="aux_ps", bufs=1, space="PSUM") as aux_ps_pool,
        ):
            # ---------------- constants ----------------
            ident_f = cpool.tile([128, 128], f32, tag="identf")
            nc.sync.dma_start(ident_f[:], ident_f32_dram[:, :])
            ident_b = cpool.tile([128, 128], bf16, tag="identb")
            nc.sync.dma_start(ident_b[:], ident_bf_dram[:, :])
            ones_col = cpool.tile([128, 1], f32, tag="onesc")
            nc.sync.dma_start(ones_col[:], ones_col_dram[:, :])
            ones_row = cpool.tile([1, 128], f32, tag="onesr")
            nc.sync.dma_start(ones_row[:], ones_row_dram[:, :])

            # W1 blocks [d-block 128, U] -> bf16
            w1b = []
            for db in range(2):
                t_f = sm_pool.tile([128, U], f32, tag="w1f")
                nc.sync.dma_start(t_f[:], w1[db * 128 : (db + 1) * 128, :])
                t_b = cpool.tile([128, U], bf16, tag=f"w1b{db}")
                nc.vector.tensor_copy(t_b[:], t_f[:])
                w1b.append(t_b)

            # V blocks [u-block 128, 1] -> bf16
            vb = []
            for ub in range(2):
                t_f = sm_pool.tile([128, 1], f32, tag="vf")
                nc.sync.dma_start(t_f[:], v[ub * 128 : (ub + 1) * 128, :])
                t_b = cpool.tile([128, 1], bf16, tag=f"vb{ub}")
                nc.vector.tensor_copy(t_b[:], t_f[:])
                vb.append(t_b)

            # qT [u-block, BC] tiles
            qT = []
            for ub in range(2):
                q_sb = cpool.tile([128, BC], f32, tag=f"qT{ub}")
                nc.sync.dma_start(q_sb[:], qt_in[ub * 128 : (ub + 1) * 128, :])
                qT.append(q_sb)

            # ---------------- main loop ----------------
            for b in range(BC):
                sc_ps = sc_ps_pool.tile([128, NCOL], f32)
                featb_tiles = []
                for c in range(NCHUNK):
                    # load chunk [1024, 256] with f32->bf16 cast in the DMA (SWDGE)
                    featb = fb_pool.tile([128, NSUB * D], bf16)
                    nc.gpsimd.dma_start(
                        featb[:].rearrange("p (n d) -> p n d", d=D),
                        feats[b, c * SCHUNK : (c + 1) * SCHUNK, :].rearrange(
                            "(n p) d -> p n d", p=128
                        ),
                    )
                    featb_tiles.append(featb)

                    # transpose per d-block: ftT[db][p, n*128 + i] = feat[s=n*128+i, d=db*128+p]
                    # emitted in s-halves so mm1 on half 0 overlaps the copy of half 1
                    ftTs = []
                    for db in range(2):
                        ftT_ps = ftt_ps_pool.tile([128, SCHUNK], bf16)
                        ftT = ftt_pool.tile([128, SCHUNK], bf16)
                        for sh in range(SCHUNK // 512):
                            for n in range(sh * 4, sh * 4 + 4):
                                nc.tensor.transpose(
                                    ftT_ps[:, n * 128 : (n + 1) * 128],
                                    featb[:, n * D + db * 128 : n * D + db * 128 + 128],
                                    ident_b[:, :],
                                )
                            nc.vector.tensor_copy(
                                ftT[:, sh * 512 : (sh + 1) * 512],
                                ftT_ps[:, sh * 512 : (sh + 1) * 512],
                            )
                        ftTs.append(ftT)

                    # mm1: projT[ub] [128, 1024] += W1[db, ub].T @ ftT[db]
                    th = th_pool.tile([128, 2 * SCHUNK], bf16)
                    for ub in range(2):
                        pf_ps = pf_ps_pool.tile([128, SCHUNK], f32)
                        for sh in range(SCHUNK // 512):
                            for db in range(2):
                                nc.tensor.matmul(
                                    pf_ps[:, sh * 512 : (sh + 1) * 512],
                                    w1b[db][:, ub * 128 : (ub + 1) * 128],
                                    ftTs[db][:, sh * 512 : (sh + 1) * 512],
                                    start=(db == 0),
                                    stop=(db == 1),
                                )
                        # tanh with fused per-partition bias q[u]
                        nc.scalar.activation(
                            th[:, ub * SCHUNK : (ub + 1) * SCHUNK],
                            pf_ps[:],
                            AF.Tanh,
                            bias=qT[ub][:, b : b + 1],
                        )

                    # mm2: score columns [128,1] per s-subtile
                    for n in range(NSUB):
                        col = c * NSUB + n
                        for ub in range(2):
                            nc.tensor.matmul(
                                sc_ps[:, col : col + 1],
                                th[:, ub * SCHUNK + n * 128 : ub * SCHUNK + (n + 1) * 128],
                                vb[ub][:],
                                start=(ub == 0),
                                stop=(ub == 1),
                            )

                # exp + partial sums (fused)
                wexp = sm_pool.tile([128, NCOL], f32, tag="wexp")
                partial = sm_pool.tile([128, 1], f32, tag="part")
                nc.scalar.activation(wexp[:], sc_ps[:], AF.Exp, accum_out=partial[:])
                wexp_b = sm_pool.tile([128, NCOL], bf16, tag="wexpb")
                nc.vector.tensor_copy(wexp_b[:], wexp[:])

                # total = sum over partitions
                tot_ps = aux_ps_pool.tile([1, 1], f32, tag="aux")
                nc.tensor.matmul(tot_ps[:], partial[:], ones_col[:])
                recip = sm_pool.tile([1, 1], f32, tag="recip")
                nc.vector.reciprocal(recip[:], tot_ps[:])
                # broadcast recip to 128 partitions via ones-matmul
                rb_ps = aux_ps_pool.tile([128, 1], f32, tag="aux")
                nc.tensor.matmul(rb_ps[:], ones_row[:], recip[:])
                recip_bc = sm_pool.tile([128, 1], f32, tag="recipbc")
                nc.vector.tensor_copy(recip_bc[:], rb_ps[:])

                # attention weights out: aw = wexp * recip, transposed for DMA
                aw_n = sm_pool.tile([128, NCOL], f32, tag="awn")
                nc.vector.tensor_scalar_mul(aw_n[:], wexp[:], recip_bc[:])
                awT_ps = aux_ps_pool.tile([NCOL, 128], f32, tag="aux")
                nc.tensor.transpose(awT_ps[:], aw_n[:], ident_f[:, :])
                awT = sm_pool.tile([NCOL, 128], f32, tag="awT")
                nc.vector.tensor_copy(awT[:], awT_ps[:])
                nc.sync.dma_start(
                    aw_out[b].rearrange("(t p) one -> t (p one)", p=128), awT[:]
                )

                # ctx: accumulate w-weighted feature sums over all 32 subtiles
                ctx_ps = aux_ps_pool.tile([1, D], f32, tag="aux")
                for c in range(NCHUNK):
                    for n in range(NSUB):
                        col = c * NSUB + n
                        nc.tensor.matmul(
                            ctx_ps[:],
                            wexp_b[:, col : col + 1],
                            featb_tiles[c][:, n * D : (n + 1) * D],
                            start=(col == 0),
                            stop=(col == NCOL - 1),
                        )
                ctxb = sm_pool.tile([1, D], f32, tag="ctxb")
                nc.vector.tensor_scalar_mul(ctxb[:], ctx_ps[:], recip[:])
                nc.sync.dma_start(ctx_out[b : b + 1, :], ctxb[:])

    nc.finalize()
    return nc


def _get_nc():
    if "nc" not in _CACHE:
        _CACHE["nc"] = _build_nc()
    return _CACHE["nc"]


def run(inputs, trace=False):
    """Run on 8 cores. Returns (ctx [B,D], aw [B,S,1], BassKernelResults)."""
    from concourse import bass_utils

    nc = _get_nc()
    np_in = {k: np.ascontiguousarray(np.asarray(v, dtype=np.float32)) for k, v in inputs.items()}
    # host-side tiny prep: qT = (hidden @ W2 + b1 + b2).T per core
    q_full = (
        np_in["hidden"] @ np_in["W2"]
        + np_in["b2"][None, :]
        + np_in["b1"][None, :]
    ).astype(np.float32)  # [B, U]
    in_maps = []
    for i in range(NCORES):
        in_maps.append(
            {
                "features": np_in["features"][i * BC : (i + 1) * BC],
                "qT": np.ascontiguousarray(q_full[i * BC : (i + 1) * BC].T),
                "W1": np_in["W1"],
                "V": np_in["V"],
            }
        )
    res = bass_utils.run_bass_kernel_spmd(
        nc, in_maps, core_ids=list(range(NCORES)), trace=trace
    )
    ctx = np.concatenate([res.results[i]["ctx"] for i in range(NCORES)], axis=0)
    aw = np.concatenate([res.results[i]["aw"] for i in range(NCORES)], axis=0)
    return ctx, aw, res


def kernel(**inputs):
    ctx, aw, _ = run(inputs, trace=False)
    return ctx, aw


# revision 34
# speedup vs baseline: 1.0117x; 1.0117x over previous
"""Bahdanau attention Trainium2 kernel.

Computation (per batch b):
  q[u]     = (hidden[b] @ W2)[u] + b2[u] + b1[u]
  projT    = W1.T @ feat[b].T            -> [U, S]   (PE, bf16)
  t        = tanh(projT + q[:, None])    -> [U, S]   (ACT, fused per-partition bias)
  score[s] = sum_u V[u] * t[u, s]                    (PE, tanh-slices as stationary)
  w[s]     = exp(score[s])               (no max-sub: |score| <= sum|V| ~ 13, safe in f32)
  total    = sum_s w[s]                              (ACT accum_out + ones-matmul)
  aw       = w / total
  ctx[d]   = sum_s w[s] * feat[b,s,d] / total        (PE, w-columns as stationary)

Data-parallel over 8 NeuronCores: batch dim 64 -> 8 per core, weights replicated.
"""

import os
import numpy as np

B, S, D, H, U = 64, 4096, 256, 512, 256
NCORES = 8
BC = B // NCORES          # batches per core
SCHUNK = 1024             # s rows per chunk
NCHUNK = S // SCHUNK      # 4
NSUB = SCHUNK // 128      # 8 sub-tiles of 128 rows per chunk
NCOL = S // 128           # 32 score columns per batch

_CACHE = {}


def _build_nc():
    import concourse.bass as bass  # noqa: F401
    import concourse.mybir as mybir
    from concourse import bacc, tile
    import ml_dtypes

    f32 = mybir.dt.float32
    bf16 = mybir.dt.bfloat16
    AF = mybir.ActivationFunctionType

    nc = bacc.Bacc("TRN2")

    feats = nc.declare_dram_parameter("features", [BC, S, D], f32, isOutput=False)
    # qT = (hidden @ W2 + b1 + b2).T, precomputed host-side (tiny)
    qt_in = nc.declare_dram_parameter("qT", [U, BC], f32, isOutput=False)
    w1 = nc.declare_dram_parameter("W1", [D, U], f32, isOutput=False)
    v = nc.declare_dram_parameter("V", [U, 1], f32, isOutput=False)
    ctx_out = nc.declare_dram_parameter("ctx", [BC, D], f32, isOutput=True)
    aw_out = nc.declare_dram_parameter("aw", [BC, S, 1], f32, isOutput=True)

    ident_f32_dram = nc.inline_tensor(np.eye(128, dtype=np.float32), name="ident_f32")
    ident_bf_dram = nc.inline_tensor(
        np.eye(128).astype(ml_dtypes.bfloat16), name="ident_bf"
    )
    ones_col_dram = nc.inline_tensor(np.ones((128, 1), np.float32), name="ones_col")
    ones_row_dram = nc.inline_tensor(np.ones((1, 128), np.float32), name="ones_row")

    with tile.TileContext(nc) as tc:
        with (
            tc.tile_pool(name="const", bufs=1) as cpool,
            tc.tile_pool(name="fb", bufs=NCHUNK + 4) as fb_pool,
            tc.tile_pool(name="ftt", bufs=8) as ftt_pool,
            tc.tile_pool(name="th", bufs=6) as th_pool,
            tc.tile_pool(name="small", bufs=3) as sm_pool,
            tc.tile_pool(name="ftt_ps", bufs=2, space="PSUM") as ftt_ps_pool,
            tc.tile_pool(name="pf_ps", bufs=2, space="PSUM") as pf_ps_pool,
            tc.tile_pool(name="sc_ps", bufs=1, space="PSUM") as sc_ps_pool,
            tc.tile_pool(name="aux_ps", bufs=1, space="PSUM") as aux_ps_pool,
        ):
            # ---------------- constants ----------------
            ident_f = cpool.tile([128, 128], f32, tag="identf")
            nc.sync.dma_start(ident_f[:], ident_f32_dram[:, :])
            ident_b = cpool.tile([128, 128], bf16, tag="identb")
            nc.sync.dma_start(ident_b[:], ident_bf_dram[:, :])
            ones_col = cpool.tile([128, 1], f32, tag="onesc")
            nc.sync.dma_start(ones_col[:], ones_col_dram[:, :])
            ones_row = cpool.tile([1, 128], f32, tag="onesr")
            nc.sync.dma_start(ones_row[:], ones_row_dram[:, :])

            # W1 blocks [d-block 128, U] -> bf16
            w1b = []
            for db in range(2):
                t_f = sm_pool.tile([128, U], f32, tag="w1f")
                nc.sync.dma_start(t_f[:], w1[db * 128 : (db + 1) * 128, :])
                t_b = cpool.tile([128, U], bf16, tag=f"w1b{db}")
                nc.vector.tensor_copy(t_b[:], t_f[:])
                w1b.append(t_b)

            # V blocks [u-block 128, 1] -> bf16
            vb = []
            for ub in range(2):
                t_f = sm_pool.tile([128, 1], f32, tag="vf")
                nc.sync.dma_start(t_f[:], v[ub * 128 : (ub + 1) * 128, :])
                t_b = cpool.tile([128, 1], bf16, tag=f"vb{ub}")
                nc.vector.tensor_copy(t_b[:], t_f[:])
                vb.append(t_b)

            # qT [u-block, BC] tiles
            qT = []
            for ub in range(2):
                q_sb = cpool.tile([128, BC], f32, tag=f"qT{ub}")
                nc.sync.dma_start(q_sb[:], qt_in[ub * 128 : (ub + 1) * 128, :])
                qT.append(q_sb)

            # ---------------- main loop ----------------
            for b in range(BC):
                sc_ps = sc_ps_pool.tile([128, NCOL], f32)
                featb_tiles = []
                for c in range(NCHUNK):
                    # load chunk [1024, 256] with f32->bf16 cast in the DMA (SWDGE)
                    featb = fb_pool.tile([128, NSUB * D], bf16)
                    for h in range(2):
                        nc.gpsimd.dma_start(
                            featb[:, h * (NSUB // 2) * D : (h + 1) * (NSUB // 2) * D]
                            .rearrange("p (n d) -> p n d", d=D),
                            feats[
                                b,
                                c * SCHUNK + h * (SCHUNK // 2) : c * SCHUNK
                                + (h + 1) * (SCHUNK // 2),
                                :,
                            ].rearrange("(n p) d -> p n d", p=128),
                        )
                    featb_tiles.append(featb)

                    # transpose per d-block: ftT[db][p, n*128 + i] = feat[s=n*128+i, d=db*128+p]
                    # emitted in s-halves so mm1 on half 0 overlaps the copy of half 1
                    ftTs = []
                    for db in range(2):
                        ftT_ps = ftt_ps_pool.tile([128, SCHUNK], bf16)
                        ftT = ftt_pool.tile([128, SCHUNK], bf16)
                        for sh in range(SCHUNK // 512):
                            for n in range(sh * 4, sh * 4 + 4):
                                nc.tensor.transpose(
                                    ftT_ps[:, n * 128 : (n + 1) * 128],
                                    featb[:, n * D + db * 128 : n * D + db * 128 + 128],
                                    ident_b[:, :],
                                )
                            nc.vector.tensor_copy(
                                ftT[:, sh * 512 : (sh + 1) * 512],
                                ftT_ps[:, sh * 512 : (sh + 1) * 512],
                            )
                        ftTs.append(ftT)

                    # mm1: projT[ub] [128, 1024] += W1[db, ub].T @ ftT[db]
                    th = th_pool.tile([128, 2 * SCHUNK], bf16)
                    for ub in range(2):
                        pf_ps = pf_ps_pool.tile([128, SCHUNK], f32)
                        for sh in range(SCHUNK // 512):
                            for db in range(2):
                                nc.tensor.matmul(
                                    pf_ps[:, sh * 512 : (sh + 1) * 512],
                                    w1b[db][:, ub * 128 : (ub + 1) * 128],
                                    ftTs[db][:, sh * 512 : (sh + 1) * 512],
                                    start=(db == 0),
                                    stop=(db == 1),
                                )
                        # tanh with fused per-partition bias q[u]
                        nc.scalar.activation(
                            th[:, ub * SCHUNK : (ub + 1) * SCHUNK],
                            pf_ps[:],
                            AF.Tanh,
                            bias=qT[ub][:, b : b + 1],
                        )

                    # mm2: score columns [128,1] per s-subtile
                    for n in range(NSUB):
                        col = c * NSUB + n
                        for ub in range(2):
                            nc.tensor.matmul(
                                sc_ps[:, col : col + 1],
                                th[:, ub * SCHUNK + n * 128 : ub * SCHUNK + (n + 1) * 128],
                                vb[ub][:],
                                start=(ub == 0),
                                stop=(ub == 1),
                            )

                # exp + partial sums (fused)
                wexp = sm_pool.tile([128, NCOL], f32, tag="wexp")
                partial = sm_pool.tile([128, 1], f32, tag="part")
                nc.scalar.activation(wexp[:], sc_ps[:], AF.Exp, accum_out=partial[:])
                wexp_b = sm_pool.tile([128, NCOL], bf16, tag="wexpb")
                nc.vector.tensor_copy(wexp_b[:], wexp[:])

                # total = sum over partitions
                tot_ps = aux_ps_pool.tile([1, 1], f32, tag="aux")
                nc.tensor.matmul(tot_ps[:], partial[:], ones_col[:])
                recip = sm_pool.tile([1, 1], f32, tag="recip")
                nc.vector.reciprocal(recip[:], tot_ps[:])
                # broadcast recip to 128 partitions via ones-matmul
                rb_ps = aux_ps_pool.tile([128, 1], f32, tag="aux")
                nc.tensor.matmul(rb_ps[:], ones_row[:], recip[:])
                recip_bc = sm_pool.tile([128, 1], f32, tag="recipbc")
                nc.vector.tensor_copy(recip_bc[:], rb_ps[:])

                # attention weights out: aw = wexp * recip, transposed for DMA
                aw_n = sm_pool.tile([128, NCOL], f32, tag="awn")
                nc.vector.tensor_scalar_mul(aw_n[:], wexp[:], recip_bc[:])
                awT_ps = aux_ps_pool.tile([NCOL, 128], f32, tag="aux")
                nc.tensor.transpose(awT_ps[:], aw_n[:], ident_f[:, :])
                awT = sm_pool.tile([NCOL, 128], f32, tag="awT")
                nc.vector.tensor_copy(awT[:], awT_ps[:])
                nc.sync.dma_start(
                    aw_out[b].rearrange("(t p) one -> t (p one)", p=128), awT[:]
                )

                # ctx: accumulate w-weighted feature sums over all 32 subtiles
                ctx_ps = aux_ps_pool.tile([1, D], f32, tag="aux")
                for c in range(NCHUNK):
                    for n in range(NSUB):
                        col = c * NSUB + n
                        nc.tensor.matmul(
                            ctx_ps[:],
                            wexp_b[:, col : col + 1],
                            featb_tiles[c][:, n * D : (n + 1) * D],
                            start=(col == 0),
                            stop=(col == NCOL - 1),
                        )
                ctxb = sm_pool.tile([1, D], f32, tag="ctxb")
                nc.vector.tensor_scalar_mul(ctxb[:], ctx_ps[:], recip[:])
                nc.sync.dma_start(ctx_out[b : b + 1, :], ctxb[:])

    nc.finalize()
    return nc


def _get_nc():
    if "nc" not in _CACHE:
        _CACHE["nc"] = _build_nc()
    return _CACHE["nc"]


def run(inputs, trace=False):
    """Run on 8 cores. Returns (ctx [B,D], aw [B,S,1], BassKernelResults)."""
    from concourse import bass_utils

    nc = _get_nc()
    np_in = {k: np.ascontiguousarray(np.asarray(v, dtype=np.float32)) for k, v in inputs.items()}
    # host-side tiny prep: qT = (hidden @ W2 + b1 + b2).T per core
    q_full = (
        np_in["hidden"] @ np_in["W2"]
        + np_in["b2"][None, :]
        + np_in["b1"][None, :]
    ).astype(np.float32)  # [B, U]
    in_maps = []
    for i in range(NCORES):
        in_maps.append(
            {
                "features": np_in["features"][i * BC : (i + 1) * BC],
                "qT": np.ascontiguousarray(q_full[i * BC : (i + 1) * BC].T),
                "W1": np_in["W1"],
                "V": np_in["V"],
            }
        )
    res = bass_utils.run_bass_kernel_spmd(
        nc, in_maps, core_ids=list(range(NCORES)), trace=trace
    )
    ctx = np.concatenate([res.results[i]["ctx"] for i in range(NCORES)], axis=0)
    aw = np.concatenate([res.results[i]["aw"] for i in range(NCORES)], axis=0)
    return ctx, aw, res


def kernel(**inputs):
    ctx, aw, _ = run(inputs, trace=False)
    return ctx, aw
